# revision 1
# baseline (speedup 1.0000x reference)
"""Dilated correlation kernel for Trainium2 (8 NeuronCores, batch-parallel).

Computes, for feat_curr/feat_prev_warped [B=8, C=256, H=128, W=192] fp32:
    out[b, o, y, x] = sum_c curr_n[b,c,y,x] * prev_n[b,c,y+dy_o,x+dx_o]
over 33 (dx, dy) offsets (radius 4, dilation 2), with L2-normalized
features and zero padding outside the image.

Strategy (per core; batch b = core id):
  - L2 normalization input-side: squares (GPSIMD, bf16 out) -> ones-matmul
    partition reduction (PE, bf16) -> 1/norm via exp(-0.5*ln(norm^2))
    (ACT) -> scale (DVE/ACT), writing bf16 normalized features.
  - Normalized prev kept as a full zero-padded plane in SBUF:
    [128c, chunk, slot=y+4, 4+W+4].
  - Correlation: banded matmuls, bf16 inputs, fp32 PSUM. Output pixels are
    tiled 128 at a time (4 col-groups of 32; each 32-run lies in one image
    row). Per col-group the rhs window is re-based, which shears the
    needed diagonal into a 32-wide window. PSUM tile [128, 512] (one bank;
    7 dy-bands x 40 packed in [0, 280)).
  - Extraction: multiply by a constant one-hot mask delta(j == p%32)
    (broadcast over offsets) then a strided windowed add-reduce (DVE).
  - Output: PE-transpose each [128px, 33] result to [33, 128px], batch 12
    tiles in SBUF, store contiguous [33, 1536px] chunks to a [33, H, W]
    DRAM layout; host permutes offsets into reference order.
"""

import os
import sys

import numpy as np

_TRN_REPO = "/opt/trn_rl_repo"
if _TRN_REPO not in sys.path:
    sys.path.insert(0, _TRN_REPO)

from contextlib import ExitStack

import concourse.bacc as bacc
import concourse.bass as bass
import concourse.mybir as mybir
import concourse.tile as tile
from concourse.bass_utils import run_bass_kernel_spmd

F32 = mybir.dt.float32
BF16 = mybir.dt.bfloat16

C, H, W = 256, 128, 192
NCORES = 8
NCHUNK = C // 128
PAD = 4
SLOT_W = W + 2 * PAD          # 200
NDYB = 7                      # dy bands, order: [-4,-2,0,2,4,-1,1]
WIN = 40                      # 32 (col-group shear span) + 8 (dx span)
EVEN_DYS = (-4, -2, 0, 2, 4)
ODD_DYS = (-1, 1)
NCOL = 33
SCAN = NCOL * 32              # 1056
OBATCH = 12                   # output tiles per store

# column order produced on device (dy, dx):
MY_OFFSETS = (
    [(dy, dx) for dy in EVEN_DYS for dx in EVEN_DYS]
    + [(dy, dx) for dy in ODD_DYS for dx in (-1, 0, 1)]
    + [(0, dx) for dx in (-1, 1)]
)


def _ref_offsets(radius=4, step=2):
    offs = []
    for dy in range(-radius, radius + 1):
        for dx in range(-radius, radius + 1):
            if abs(dx) <= 1 and abs(dy) <= 1:
                offs.append((dx, dy))
                continue
            if abs(dx) % step == 0 and abs(dy) % step == 0:
                offs.append((dx, dy))
    return offs


# perm[o_ref] = device column holding reference offset o_ref
PERM = np.array(
    [MY_OFFSETS.index((dy, dx)) for (dx, dy) in _ref_offsets()], dtype=np.int64
)


def make_dmask():
    m = np.zeros((128, 32), dtype=np.float32)
    for p in range(128):
        m[p, p % 32] = 1.0
    return m


def make_ident():
    return np.eye(128, dtype=np.float32)


def build_nc(h=H, loop_k=0):
    nslot = h + 2 * PAD
    nt = (h * W) // 128
    assert nt % OBATCH == 0
    nc = bacc.Bacc()
    curr_d = nc.declare_dram_parameter("curr", [C, h, W], F32, isOutput=False)
    prev_d = nc.declare_dram_parameter("prev", [C, h, W], F32, isOutput=False)
    mask_d = nc.declare_dram_parameter("dmask", [128, 32], F32, isOutput=False)
    id_d = nc.declare_dram_parameter("ident", [128, 128], F32, isOutput=False)
    out_d = nc.declare_dram_parameter("out", [NCOL, h, W], F32, isOutput=True)

    with tile.TileContext(nc) as tc, ExitStack() as ctx:
        cpool = ctx.enter_context(tc.tile_pool(name="const", bufs=1))
        ldpool = ctx.enter_context(tc.tile_pool(name="ld", bufs=4))
        sqpool = ctx.enter_context(tc.tile_pool(name="sq", bufs=2))
        lnpool = ctx.enter_context(tc.tile_pool(name="lnp", bufs=2))
        rnpool = ctx.enter_context(tc.tile_pool(name="rn", bufs=3))
        cnpool = ctx.enter_context(tc.tile_pool(name="cn", bufs=10))
        zbpool = ctx.enter_context(tc.tile_pool(name="zb", bufs=2))
        outpool = ctx.enter_context(tc.tile_pool(name="outp", bufs=4))
        obufpool = ctx.enter_context(tc.tile_pool(name="obuf", bufs=2))
        pscpool = ctx.enter_context(tc.tile_pool(name="psc", bufs=4, space="PSUM"))
        psnpool = ctx.enter_context(tc.tile_pool(name="psn", bufs=2, space="PSUM"))
        pstpool = ctx.enter_context(tc.tile_pool(name="pst", bufs=2, space="PSUM"))

        plane = cpool.tile([128, NCHUNK, nslot, SLOT_W], BF16, name="plane")
        ones = cpool.tile([128, 128], BF16, name="ones")
        dmask = cpool.tile([128, 32], F32, name="dmask")
        ident = cpool.tile([128, 128], F32, name="ident")
        nc.gpsimd.memset(plane[:], 0.0)
        nc.gpsimd.memset(ones[:], 1.0)
        nc.sync.dma_start(dmask[:], mask_d[:])
        nc.sync.dma_start(ident[:], id_d[:])

        cn = {}     # row -> curr normalized bf16 [128, NCHUNK, W]
        state = {}  # mutable emission state

        def normalize_rows2(ld, rg, r2):
            """normalize rows rg+r2 .. rg+r2+1 from the 8-row load tiles"""
            for nm in ("c", "p"):
                sq = sqpool.tile(
                    [128, NCHUNK, 2 * W], BF16, name=f"sq_{nm}_{rg}_{r2}", tag="sq"
                )
                for ch in range(NCHUNK):
                    nc.gpsimd.tensor_tensor(
                        sq[:, ch, :],
                        ld[(nm, ch)][:, r2 : r2 + 2, :].rearrange("p a b -> p (a b)"),
                        ld[(nm, ch)][:, r2 : r2 + 2, :].rearrange("p a b -> p (a b)"),
                        mybir.AluOpType.mult,
                    )
                psn = psnpool.tile(
                    [128, 2 * W], F32, name=f"psn_{nm}_{rg}_{r2}", tag="psn"
                )
                for ch in range(NCHUNK):
                    nc.tensor.matmul(
                        psn[:],
                        ones[:],
                        sq[:, ch, :],
                        start=(ch == 0),
                        stop=(ch == NCHUNK - 1),
                    )
                lnr = lnpool.tile(
                    [128, 2 * W], F32, name=f"ln_{nm}_{rg}_{r2}", tag="lnr"
                )
                nc.scalar.activation(lnr[:], psn[:], mybir.ActivationFunctionType.Ln)
                rn = rnpool.tile([128, 2 * W], F32, name=f"rn_{nm}_{rg}_{r2}", tag="rn")
                nc.scalar.activation(
                    rn[:], lnr[:], mybir.ActivationFunctionType.Exp, scale=-0.5
                )
                for dr in range(2):
                    r = rg + r2 + dr
                    if nm == "p":
                        for ch in range(NCHUNK):
                            nc.vector.tensor_mul(
                                plane[:, ch, r + PAD, PAD : PAD + W],
                                ld[("p", ch)][:, r2 + dr, :],
                                rn[:, dr * W : (dr + 1) * W],
                            )
                    else:
                        t = cnpool.tile(
                            [128, NCHUNK, W], BF16, name=f"cn_{r}", tag="cn"
                        )
                        for ch in range(NCHUNK):
                            nc.vector.tensor_mul(
                                t[:, ch, :],
                                ld[("c", ch)][:, r2 + dr, :],
                                rn[:, dr * W : (dr + 1) * W],
                            )
                        cn[r] = t

        def emit_tile(t):
            # one full PSUM bank per partition: bands packed contiguously at
            # 40 cols each in [0, 280); the 512 stride keeps every
            # col-group's flat offset bank-aligned
            PSTR = 512
            ps = pscpool.tile([128, PSTR], F32, name=f"ps_{t}", tag="ps")
            pst = ps.tensor
            for g in range(4):
                q = 128 * t + 32 * g
                r, x0 = divmod(q, W)
                lhs = cn[r]
                for dyb0, ndy, s0 in ((0, 5, r), (5, 2, r + 3)):
                    for ch in range(NCHUNK):
                        rhs = plane[:, ch, s0 : s0 + 2 * ndy - 1 : 2, x0 : x0 + WIN]
                        out_ap = bass.AP(
                            pst,
                            32 * g * PSTR + dyb0 * WIN,
                            [[PSTR, 32], [1, ndy * WIN]],
                        )
                        nc.tensor.matmul(
                            out_ap,
                            lhs[:, ch, x0 : x0 + 32],
                            rhs,
                            start=(ch == 0),
                            stop=(ch == NCHUNK - 1),
                            tile_position=(0, 32 * g),
                        )
            zb = zbpool.tile([128, SCAN], F32, name=f"zb_{t}", tag="zb")
            zbt, dmt = zb.tensor, dmask.tensor
            # class i: dy in evens (bands 0-4), dx in evens
            nc.any.tensor_mul(
                bass.AP(zbt, 0, [[SCAN, 128], [160, 5], [32, 5], [1, 32]]),
                bass.AP(pst, 0, [[PSTR, 128], [WIN, 5], [2, 5], [1, 32]]),
                bass.AP(dmt, 0, [[32, 128], [0, 5], [0, 5], [1, 32]]),
            )
            # class ii: dy in {-1,+1} (bands 5,6), dx in {-1,0,1}
            nc.any.tensor_mul(
                bass.AP(zbt, 800, [[SCAN, 128], [96, 2], [32, 3], [1, 32]]),
                bass.AP(pst, 5 * WIN + 3, [[PSTR, 128], [WIN, 2], [1, 3], [1, 32]]),
                bass.AP(dmt, 0, [[32, 128], [0, 2], [0, 3], [1, 32]]),
            )
            # class iii: dy=0 (band 2), dx in {-1,+1}
            nc.any.tensor_mul(
                bass.AP(zbt, 992, [[SCAN, 128], [32, 2], [1, 32]]),
                bass.AP(pst, 2 * WIN + 3, [[PSTR, 128], [2, 2], [1, 32]]),
                bass.AP(dmt, 0, [[32, 128], [0, 2], [1, 32]]),
            )
            outt = outpool.tile([128, 40], F32, name=f"out_{t}", tag="outt")
            nc.vector.tensor_reduce(
                outt[:, 0:NCOL],
                bass.AP(zbt, 0, [[SCAN, 128], [32, NCOL], [1, 32]]),
                axis=mybir.AxisListType.X,
                op=mybir.AluOpType.add,
            )
            # transpose to [33, 128] and batch into obuf for contiguous stores
            k = t % OBATCH
            if k == 0:
                state["obuf"] = obufpool.tile(
                    [40, OBATCH * 128], F32, name=f"obuf_{t}", tag="obuf"
                )
            tps = pstpool.tile([40, 128], F32, name=f"tps_{t}", tag="tps")
            nc.tensor.transpose(tps[:], outt[:], ident[:])
            nc.scalar.copy(
                state["obuf"][0:NCOL, 128 * k : 128 * (k + 1)], tps[0:NCOL, :]
            )
            if k == OBATCH - 1:
                t0 = t - (OBATCH - 1)
                nc.sync.dma_start(
                    bass.AP(
                        out_d,
                        128 * t0,
                        [[h * W, NCOL], [1, OBATCH * 128]],
                    ),
                    state["obuf"][0:NCOL, :],
                )

        def whole_body():
            cn.clear()
            next_t = 0
            for rg in range(0, h, 8):
                ld = {}
                for nm, dram in (("c", curr_d), ("p", prev_d)):
                    for ch in range(NCHUNK):
                        t = ldpool.tile(
                            [128, 8, W], F32, name=f"ld_{nm}{ch}_{rg}", tag="ld"
                        )
                        nc.sync.dma_start(
                            t[:], dram[ch * 128 : (ch + 1) * 128, rg : rg + 8, :]
                        )
                        ld[(nm, ch)] = t
                for r2 in (0, 2, 4, 6):
                    normalize_rows2(ld, rg, r2)
                    r_done = rg + r2 + 1
                    while next_t < nt and (128 * next_t + 96) // W + PAD <= r_done:
                        emit_tile(next_t)
                        next_t += 1
            while next_t < nt:
                emit_tile(next_t)
                next_t += 1

        if loop_k:
            with tc.For_i(0, loop_k, 1):
                whole_body()
        else:
            whole_body()

    nc.finalize()
    return nc


_NC_CACHE = {}
LAST_EXEC_NS = None


def _get_nc(h=H):
    if h not in _NC_CACHE:
        _NC_CACHE[h] = build_nc(h)
    return _NC_CACHE[h]


def kernel(feat_curr: np.ndarray, feat_prev_warped: np.ndarray) -> np.ndarray:
    global LAST_EXEC_NS
    feat_curr = np.ascontiguousarray(np.asarray(feat_curr, dtype=np.float32))
    feat_prev_warped = np.ascontiguousarray(
        np.asarray(feat_prev_warped, dtype=np.float32)
    )
    b, c, h, w = feat_curr.shape
    assert (b, c, w) == (NCORES, C, W), (b, c, w)

    nc = _get_nc(h)
    dmask = make_dmask()
    ident = make_ident()
    in_maps = [
        {
            "curr": feat_curr[i],
            "prev": feat_prev_warped[i],
            "dmask": dmask,
            "ident": ident,
        }
        for i in range(NCORES)
    ]
    res = run_bass_kernel_spmd(nc, in_maps, list(range(NCORES)))
    LAST_EXEC_NS = res.exec_time_ns
    out = np.stack([res.results[i]["out"] for i in range(NCORES)])  # [B, 33, H, W]
    out = out[:, PERM]  # reference offset order
    return np.ascontiguousarray(out)


def time_kernel(
    inputs_np: dict, n_iters: int = 10, k_lo: int = 8, k_hi: int = 136
) -> int:
    """Estimate per-iteration HW time by differencing two on-device-looped
    variants of the kernel (axon dispatch floor ~80ms makes single-shot wall
    timing useless)."""
    lo = _time_nc(build_nc(H, loop_k=k_lo), inputs_np, n_iters)
    hi = _time_nc(build_nc(H, loop_k=k_hi), inputs_np, n_iters)
    return max(0, int(round((hi - lo) / (k_hi - k_lo))))


def _time_nc(nc, inputs_np: dict, n_iters: int = 10) -> int:
    """Min wall-clock ns over n_iters of the jitted sharded executable with
    device-resident inputs (jit'd once; donated output buffers re-placed
    untimed before each run)."""
    import time

    import jax
    from jax.experimental.shard_map import shard_map
    from jax.sharding import Mesh, PartitionSpec

    from concourse import bass2jax

    bass2jax.install_neuronx_cc_hook()

    feat_curr = np.asarray(inputs_np["feat_curr"], dtype=np.float32)
    feat_prev = np.asarray(inputs_np["feat_prev_warped"], dtype=np.float32)

    partition_name = nc.partition_id_tensor.name if nc.partition_id_tensor else None
    in_names, out_names, out_avals, zero_outs = [], [], [], []
    for alloc in nc.m.functions[0].allocations:
        if not isinstance(alloc, mybir.MemoryLocationSet):
            continue
        name = alloc.memorylocations[0].name
        if alloc.kind == "ExternalInput":
            if name != partition_name:
                in_names.append(name)
        elif alloc.kind == "ExternalOutput":
            out_names.append(name)
            shape = tuple(alloc.tensor_shape)
            dtype = mybir.dt.np(alloc.dtype)
            out_avals.append(jax.core.ShapedArray(shape, dtype))
            zero_outs.append(np.zeros(shape, dtype))
    n_params = len(in_names)
    n_outs = len(out_avals)
    in_names = in_names + out_names
    if partition_name is not None:
        in_names.append(partition_name)
    donate = tuple(range(n_params, n_params + n_outs))

    def _body(*args):
        operands = list(args)
        if partition_name is not None:
            operands.append(bass2jax.partition_id_tensor())
        outs = bass2jax._bass_exec_p.bind(
            *operands,
            out_avals=tuple(out_avals),
            in_names=tuple(in_names),
            out_names=tuple(out_names),
            lowering_input_output_aliases=(),
            sim_require_finite=True,
            sim_require_nnan=True,
            nc=nc,
        )
        return tuple(outs)

    devices = jax.devices()[:NCORES]
    mesh = Mesh(np.asarray(devices), ("core",))
    sharded = jax.jit(
        shard_map(
            _body,
            mesh=mesh,
            in_specs=(PartitionSpec("core"),) * (n_params + n_outs),
            out_specs=(PartitionSpec("core"),) * n_outs,
            check_rep=False,
        ),
        donate_argnums=donate,
        keep_unused=True,
    )
    in_map = {
        "curr": feat_curr,
        "prev": feat_prev,
        "dmask": make_dmask(),
        "ident": make_ident(),
    }
    concat_in = [
        np.concatenate(
            [
                in_map[name][c] if in_map[name].ndim == 4 else in_map[name]
                for c in range(NCORES)
            ],
            axis=0,
        )
        for name in in_names[:n_params]
    ]
    sharding = jax.sharding.NamedSharding(mesh, PartitionSpec("core"))
    dev_in = [jax.device_put(a, sharding) for a in concat_in]
    for a in dev_in:
        a.block_until_ready()

    def make_zeros():
        zs = [
            jax.device_put(
                np.zeros((NCORES * z.shape[0], *z.shape[1:]), z.dtype), sharding
            )
            for z in zero_outs
        ]
        for z in zs:
            z.block_until_ready()
        return zs

    outs = sharded(*dev_in, *make_zeros())
    for o in outs:
        o.block_until_ready()

    best = None
    for _ in range(n_iters):
        zs = make_zeros()
        t0 = time.perf_counter_ns()
        outs = sharded(*dev_in, *zs)
        for o in outs:
            o.block_until_ready()
        dt = time.perf_counter_ns() - t0
        best = dt if best is None else min(best, dt)
    return best



# revision 17
# speedup vs baseline: 1.7099x; 1.7099x over previous
"""Dilated correlation kernel for Trainium2 (8 NeuronCores, batch-parallel).

Computes, for feat_curr/feat_prev_warped [B=8, C=256, H=128, W=192] fp32:
    out[b, o, y, x] = sum_c curr_n[b,c,y,x] * prev_n[b,c,y+dy_o,x+dx_o]
over 33 (dx, dy) offsets (radius 4, dilation 2), with L2-normalized
features and zero padding outside the image.

v2 strategy (per core; batch b = core id):
  - Norms: squares (ACT Square, bf16) -> ones-matmul partition reduction
    (PE) -> sqrt (ACT, single 'sqrt' table -> no table reloads) ->
    reciprocal (DVE custom op RECIPROCAL_APPROX_FAST).
  - prev scaled by 1/||prev|| into a zero-padded bf16 plane
    [128c, ch, slot=y+4, 4+W+4]; curr kept RAW bf16 in a 16-row ring with
    16-zero gaps between 16-pixel blocks (the gaps implement half-group
    masking in the PE); curr's 1/||.|| is folded into the output stage.
  - Correlation: banded matmuls with 16-pixel half-groups. Two halves (A/B)
    of each 32-pixel PE quadrant accumulate into the SAME psum columns
    using zero-padded weights ([pix|0] / [0|pix]), so every psum partition
    sees an identically-based window and the one-hot extraction scan is
    only 16 wide. Windows: 24 cols x 5 even-dy bands, 18 cols x 2 odd-dy
    bands -> psum tile [128, 156].
  - Extraction: one-hot mask multiply (classes split DVE/Pool) + strided
    windowed add-reduce (DVE, bf16 zb).
  - Output: PE-transpose [128px, 40] -> [40, 128px]; multiply by the curr
    1/norm row (broadcast across partitions) while batching into obuf;
    contiguous [33, 1536] stores; host permutes offsets into ref order.
"""

import os
import sys

import numpy as np

_TRN_REPO = "/opt/trn_rl_repo"
if _TRN_REPO not in sys.path:
    sys.path.insert(0, _TRN_REPO)

from contextlib import ExitStack

import concourse.bacc as bacc
import concourse.bass as bass
import concourse.mybir as mybir
import concourse.tile as tile
from concourse.bass_utils import run_bass_kernel_spmd

F32 = mybir.dt.float32
BF16 = mybir.dt.bfloat16
AF = mybir.ActivationFunctionType

C, H, W = 256, 128, 192
NCORES = 8
NCHUNK = C // 128
PAD = 4
SLOT_W = W + 2 * PAD          # 200
PRING = 48                    # prev plane ring period (slots)
PECHO = 8                     # echo slots (rows with slot < PECHO double-write)
NSLOT = PRING + PECHO         # 56 physical slots
RING = 16                     # curr bf16 ring rows
G = 16                        # half-group size == one-hot scan width
EV_WIN = 24                   # even-dy band window (dy in -4,-2,0,2,4)
OD_WIN = 18                   # odd-dy band window (dy in -1,+1)
EV_COLS = 5 * EV_WIN          # 120
PS_W = EV_COLS + 2 * OD_WIN   # 156
PSTR = 512                    # corr psum tile row pitch (one full bank)
NCOL = 33
OBATCH = 12
EVEN_DYS = (-4, -2, 0, 2, 4)

# flat per-partition strides
CN_PSTR = RING * NCHUNK * 12 * 32          # cn ring row stride = 768/row
PL_PSTR = NCHUNK * NSLOT * SLOT_W          # plane partition stride
PL_CH = NSLOT * SLOT_W                     # plane chunk stride

# column order produced on device (dy, dx): identical to v1
MY_OFFSETS = (
    [(dy, dx) for dy in EVEN_DYS for dx in EVEN_DYS]
    + [(dy, dx) for dy in (-1, 1) for dx in (-1, 0, 1)]
    + [(0, dx) for dx in (-1, 1)]
)


def _ref_offsets(radius=4, step=2):
    offs = []
    for dy in range(-radius, radius + 1):
        for dx in range(-radius, radius + 1):
            if abs(dx) <= 1 and abs(dy) <= 1:
                offs.append((dx, dy))
                continue
            if abs(dx) % step == 0 and abs(dy) % step == 0:
                offs.append((dx, dy))
    return offs


# perm[o_ref] = device column holding reference offset o_ref
PERM = np.array(
    [MY_OFFSETS.index((dy, dx)) for (dx, dy) in _ref_offsets()], dtype=np.int64
)


def make_dmask():
    m = np.zeros((128, G), dtype=np.float32)
    for p in range(128):
        m[p, p % G] = 1.0
    return m


def make_ident():
    return np.eye(128, dtype=np.float32)


# engine assignment knobs ('v' = DVE, 'g' = Pool/gpsimd)
# NOTE: GPSIMD (Pool) cannot read PSUM on hardware, so anything touching the
# correlation psum tile or the transpose psum tile must run on DVE (or ACT).
ENG_MULT1 = "v"
ENG_MULT23 = "v"
ENG_PREVSCALE = "g"
ENG_OBUF = "v"


def build_nc(h=H, loop_k=0):
    assert h == H
    nt = (h * W) // 128  # 192
    assert nt % OBATCH == 0
    nc = bacc.Bacc()
    curr_d = nc.declare_dram_parameter("curr", [C, h, W], F32, isOutput=False)
    prev_d = nc.declare_dram_parameter("prev", [C, h, W], F32, isOutput=False)
    mask_d = nc.declare_dram_parameter("dmask", [128, G], F32, isOutput=False)
    id_d = nc.declare_dram_parameter("ident", [128, 128], F32, isOutput=False)
    out_d = nc.declare_dram_parameter("out", [NCOL, h, W], F32, isOutput=True)

    eng = {"v": nc.vector, "g": nc.gpsimd}

    with tile.TileContext(nc) as tc, ExitStack() as ctx:
        cpool = ctx.enter_context(tc.tile_pool(name="const", bufs=1))
        ldpool = ctx.enter_context(tc.tile_pool(name="ld", bufs=2))
        sqpool = ctx.enter_context(tc.tile_pool(name="sq", bufs=2))
        snpool = ctx.enter_context(tc.tile_pool(name="sn", bufs=2))
        rnpool = ctx.enter_context(tc.tile_pool(name="rn", bufs=4))
        zbpool = ctx.enter_context(tc.tile_pool(name="zb", bufs=2))
        outpool = ctx.enter_context(tc.tile_pool(name="outp", bufs=4))
        obufpool = ctx.enter_context(tc.tile_pool(name="obuf", bufs=2))
        pscpool = ctx.enter_context(tc.tile_pool(name="psc", bufs=2, space="PSUM"))
        psnpool = ctx.enter_context(tc.tile_pool(name="psn", bufs=2, space="PSUM"))
        pstpool = ctx.enter_context(tc.tile_pool(name="pst", bufs=2, space="PSUM"))

        plane = cpool.tile([128, NCHUNK, NSLOT, SLOT_W], BF16, name="plane")
        cnr = cpool.tile([128, RING, NCHUNK, 12, 32], BF16, name="cnr")
        ones = cpool.tile([128, 128], BF16, name="ones")
        dmask = cpool.tile([128, G], F32, name="dmask")
        identf = cpool.tile([128, 128], F32, name="identf")
        identb = cpool.tile([128, 128], BF16, name="identb")
        plt = plane.tensor
        cnrt = cnr.tensor

        # zero pads of the ring plane: top pad rows -4..-1 live at slots 0..3
        # (and their echoes 48..51); left/right column pads persist forever.
        nc.gpsimd.memset(plane[:, :, 0:PAD, :], 0.0)
        nc.gpsimd.memset(plane[:, :, PRING : PRING + PAD, :], 0.0)
        nc.gpsimd.memset(plane[:, :, :, 0:PAD], 0.0)
        nc.gpsimd.memset(plane[:, :, :, SLOT_W - PAD : SLOT_W], 0.0)
        # zero gaps of the curr ring (pixel halves get overwritten; gaps persist)
        nc.gpsimd.memset(cnr[:], 0.0)
        nc.gpsimd.memset(ones[:], 1.0)
        nc.sync.dma_start(dmask[:], mask_d[:])
        nc.sync.dma_start(identf[:], id_d[:])
        nc.scalar.activation(identb[:], identf[:], AF.Copy)

        rn = {}     # (inm, strip4) -> [128, 768] f32, bcast over partitions
        state = {}

        def process_rowgroup(rg):
            ld = {}
            for ch in range(NCHUNK):
                t_ld = ldpool.tile(
                    [128, 2, 8, W], F32, name=f"ld{ch}_{rg}", tag=f"ld{ch}"
                )
                for inm, dram in ((0, curr_d), (1, prev_d)):
                    nc.sync.dma_start(
                        t_ld[:, inm], dram[ch * 128 : (ch + 1) * 128, rg : rg + 8, :]
                    )
                ld[ch] = t_ld
            # squares, both names in one op per chunk
            sq = {}
            for ch in range(NCHUNK):
                s = sqpool.tile([128, 2, 8 * W], BF16, name=f"sq{ch}_{rg}", tag=f"sq{ch}")
                nc.scalar.activation(
                    s[:],
                    ld[ch][:].rearrange("p n r w -> p n (r w)"),
                    AF.Square,
                )
                sq[ch] = s
            # norms per (name, 4-row strip): psn -> sqrt -> reciprocal
            for inm in range(2):
                for h4 in range(2):
                    s4 = rg // 4 + h4
                    # psum tile padded to a bank multiple (1024 f32 = 2 banks);
                    # matmul writes split at the 512-col bank boundary
                    psn = psnpool.tile(
                        [128, 1024], F32, name=f"psn_{inm}_{s4}", tag="psn"
                    )
                    for c0, c1 in ((0, 512), (512, 4 * W)):
                        for ch in range(NCHUNK):
                            nc.tensor.matmul(
                                psn[:, c0:c1],
                                ones[:],
                                sq[ch][:, inm, h4 * 4 * W + c0 : h4 * 4 * W + c1],
                                start=(ch == 0),
                                stop=(ch == NCHUNK - 1),
                            )
                    sn = snpool.tile([128, 4 * W], F32, name=f"sn_{inm}_{s4}", tag="sn")
                    nc.scalar.activation(sn[:], psn[:, 0 : 4 * W], AF.Sqrt)
                    rn_t = rnpool.tile(
                        [128, 4 * W], F32, name=f"rn_{inm}_{s4}", tag=f"rn{inm}"
                    )
                    nc.vector.reciprocal_approx_fast(rn_t[:], sn[:])
                    rn[(inm, s4)] = rn_t
            # prev scale into ring plane; curr raw bf16 into ring (zero-gap layout)
            for ch in range(NCHUNK):
                for h4 in range(2):
                    s4 = rg // 4 + h4
                    r0 = rg + 4 * h4
                    sP = (r0 + PAD) % PRING
                    out_ap = bass.AP(
                        plt,
                        ch * PL_CH + sP * SLOT_W + PAD,
                        [[PL_PSTR, 128], [SLOT_W, 4], [1, W]],
                    )
                    eng[ENG_PREVSCALE].tensor_mul(
                        out_ap,
                        ld[ch][:, 1, 4 * h4 : 4 * h4 + 4, :],
                        rn[(1, s4)][:].rearrange("p (r w) -> p r w", w=W),
                    )
                    if sP < PECHO:
                        # echo into slots sP+PRING so windows that wrap read
                        # a contiguous slot range
                        nc.scalar.activation(
                            bass.AP(
                                plt,
                                ch * PL_CH + (sP + PRING) * SLOT_W + PAD,
                                [[PL_PSTR, 128], [SLOT_W, 4], [1, W]],
                            ),
                            out_ap,
                            AF.Copy,
                        )
                cn_out = bass.AP(
                    cnrt,
                    (rg % RING) * NCHUNK * 384 + ch * 384 + 16,
                    [[CN_PSTR, 128], [NCHUNK * 384, 8], [32, 12], [1, 16]],
                )
                nc.scalar.activation(
                    cn_out,
                    ld[ch][:, 0].rearrange("p r (b j) -> p r b j", j=16),
                    AF.Copy,
                )

        def emit_tile(t):
            # padded to one full bank so pool packing keeps matmul writes
            # bank-aligned; only the first PS_W columns are used
            ps = pscpool.tile([128, 512], F32, name=f"ps_{t}", tag="ps")
            pst = ps.tensor
            for q in range(4):
                qpix = 128 * t + 32 * q
                r, x0 = divmod(qpix, W)
                kA = x0 // 16
                sB = r % PRING
                # the evens group must close before the odds group opens:
                # both live in the same psum bank and `start` zeroes the
                # whole pending-zero region
                for region in range(2):
                    for ch in range(NCHUNK):
                        for hh in range(2):
                            lhs = bass.AP(
                                cnrt,
                                (r % RING) * NCHUNK * 384
                                + ch * 384
                                + 32 * kA
                                + 16
                                + 16 * hh,
                                [[CN_PSTR, 128], [1, 32]],
                            )
                            x0h = x0 + 16 * hh
                            first = ch == 0 and hh == 0
                            last = ch == NCHUNK - 1 and hh == 1
                            if region == 0:
                                rhs = bass.AP(
                                    plt,
                                    ch * PL_CH + sB * SLOT_W + x0h,
                                    [[PL_PSTR, 128], [2 * SLOT_W, 5], [1, EV_WIN]],
                                )
                                out_ap = bass.AP(
                                    pst, 32 * q * PSTR, [[PSTR, 32], [1, EV_COLS]]
                                )
                            else:
                                rhs = bass.AP(
                                    plt,
                                    ch * PL_CH + (sB + 3) * SLOT_W + x0h + 3,
                                    [[PL_PSTR, 128], [2 * SLOT_W, 2], [1, OD_WIN]],
                                )
                                out_ap = bass.AP(
                                    pst,
                                    32 * q * PSTR + EV_COLS,
                                    [[PSTR, 32], [1, 2 * OD_WIN]],
                                )
                            nc.tensor.matmul(
                                out_ap,
                                lhs,
                                rhs,
                                start=first,
                                stop=last,
                                tile_position=(0, 32 * q),
                            )
            zb = zbpool.tile([128, NCOL * G], BF16, name=f"zb_{t}", tag="zb")
            zbt, dmt = zb.tensor, dmask.tensor
            SC = NCOL * G
            # class i: dy even (bands 0-4), dx even
            eng[ENG_MULT1].tensor_mul(
                bass.AP(zbt, 0, [[SC, 128], [5 * G, 5], [G, 5], [1, G]]),
                bass.AP(pst, 0, [[PSTR, 128], [EV_WIN, 5], [2, 5], [1, G]]),
                bass.AP(dmt, 0, [[G, 128], [0, 5], [0, 5], [1, G]]),
            )
            # class ii: dy in {-1,+1} (odd bands), dx in {-1,0,1}
            eng[ENG_MULT23].tensor_mul(
                bass.AP(zbt, 25 * G, [[SC, 128], [3 * G, 2], [G, 3], [1, G]]),
                bass.AP(pst, EV_COLS, [[PSTR, 128], [OD_WIN, 2], [1, 3], [1, G]]),
                bass.AP(dmt, 0, [[G, 128], [0, 2], [0, 3], [1, G]]),
            )
            # class iii: dy=0 (band 2), dx in {-1,+1}
            eng[ENG_MULT23].tensor_mul(
                bass.AP(zbt, 31 * G, [[SC, 128], [G, 2], [1, G]]),
                bass.AP(pst, 2 * EV_WIN + 3, [[PSTR, 128], [2, 2], [1, G]]),
                bass.AP(dmt, 0, [[G, 128], [0, 2], [1, G]]),
            )
            # one-hot group reduce as a bf16 pairwise-add tree on Pool (zb is
            # SBUF so Pool may read it; at most one addend per pair is nonzero
            # so bf16 adds are exact)
            zh = zbpool.tile([128, NCOL, 8], BF16, name=f"zh_{t}", tag="zh")
            zht = zh.tensor
            outt = outpool.tile([128, 40], BF16, name=f"out_{t}", tag="outt")
            with nc.allow_low_precision("one-hot select: single-term sums"):
                nc.gpsimd.tensor_add(
                    bass.AP(zht, 0, [[NCOL * 8, 128], [8, NCOL], [1, 8]]),
                    bass.AP(zbt, 0, [[SC, 128], [G, NCOL], [1, 8]]),
                    bass.AP(zbt, 8, [[SC, 128], [G, NCOL], [1, 8]]),
                )
                nc.gpsimd.tensor_add(
                    bass.AP(zht, 4, [[NCOL * 8, 128], [8, NCOL], [1, 4]]),
                    bass.AP(zht, 0, [[NCOL * 8, 128], [8, NCOL], [1, 4]]),
                    bass.AP(zht, 4, [[NCOL * 8, 128], [8, NCOL], [1, 4]]),
                )
                nc.gpsimd.tensor_add(
                    bass.AP(zht, 6, [[NCOL * 8, 128], [8, NCOL], [1, 2]]),
                    bass.AP(zht, 4, [[NCOL * 8, 128], [8, NCOL], [1, 2]]),
                    bass.AP(zht, 6, [[NCOL * 8, 128], [8, NCOL], [1, 2]]),
                )
                nc.gpsimd.tensor_add(
                    outt[:, 0:NCOL],
                    bass.AP(zht, 6, [[NCOL * 8, 128], [8, NCOL]]),
                    bass.AP(zht, 7, [[NCOL * 8, 128], [8, NCOL]]),
                )
            k = t % OBATCH
            if k == 0:
                state["obuf"] = obufpool.tile(
                    [40, OBATCH * 128], F32, name=f"obuf_{t}", tag="obuf"
                )
            tps = pstpool.tile([40, 128], BF16, name=f"tps_{t}", tag="tps")
            nc.tensor.transpose(tps[:], outt[:], identb[:])
            # fold curr 1/norm in while batching into obuf
            s4 = t // 6
            eng[ENG_OBUF].tensor_mul(
                state["obuf"][0:NCOL, 128 * k : 128 * (k + 1)],
                tps[0:NCOL, :],
                rn[(0, s4)][0:NCOL, 128 * (t % 6) : 128 * (t % 6 + 1)],
            )
            if k == OBATCH - 1:
                t0 = t - (OBATCH - 1)
                nc.sync.dma_start(
                    bass.AP(
                        out_d,
                        128 * t0,
                        [[h * W, NCOL], [1, OBATCH * 128]],
                    ),
                    state["obuf"][0:NCOL, :],
                )

        def whole_body():
            rn.clear()
            next_t = 0
            for rg in range(0, h, 8):
                process_rowgroup(rg)
                while next_t < nt and (128 * next_t + 127) // W <= rg + 3:
                    emit_tile(next_t)
                    next_t += 1
            # bottom pad rows 128..131 at slots (128+4)%PRING..+4
            sBot = (h + PAD) % PRING
            nc.gpsimd.memset(plane[:, :, sBot : sBot + PAD, :], 0.0)
            while next_t < nt:
                emit_tile(next_t)
                next_t += 1

        if loop_k:
            with tc.For_i(0, loop_k, 1):
                whole_body()
        else:
            whole_body()

    nc.finalize()
    return nc


_NC_CACHE = {}
LAST_EXEC_NS = None


def _get_nc(h=H):
    if h not in _NC_CACHE:
        _NC_CACHE[h] = build_nc(h)
    return _NC_CACHE[h]


def kernel(feat_curr: np.ndarray, feat_prev_warped: np.ndarray) -> np.ndarray:
    global LAST_EXEC_NS
    feat_curr = np.ascontiguousarray(np.asarray(feat_curr, dtype=np.float32))
    feat_prev_warped = np.ascontiguousarray(
        np.asarray(feat_prev_warped, dtype=np.float32)
    )
    b, c, h, w = feat_curr.shape
    assert (b, c, w) == (NCORES, C, W), (b, c, w)

    nc = _get_nc(h)
    dmask = make_dmask()
    ident = make_ident()
    in_maps = [
        {
            "curr": feat_curr[i],
            "prev": feat_prev_warped[i],
            "dmask": dmask,
            "ident": ident,
        }
        for i in range(NCORES)
    ]
    res = run_bass_kernel_spmd(nc, in_maps, list(range(NCORES)))
    LAST_EXEC_NS = res.exec_time_ns
    out = np.stack([res.results[i]["out"] for i in range(NCORES)])  # [B, 33, H, W]
    out = out[:, PERM]  # reference offset order
    return np.ascontiguousarray(out)


def time_kernel(
    inputs_np: dict, n_iters: int = 10, k_lo: int = 8, k_hi: int = 136
) -> int:
    """Estimate per-iteration HW time by differencing two on-device-looped
    variants of the kernel (axon dispatch floor ~80ms makes single-shot wall
    timing useless)."""
    lo = _time_nc(build_nc(H, loop_k=k_lo), inputs_np, n_iters)
    hi = _time_nc(build_nc(H, loop_k=k_hi), inputs_np, n_iters)
    return max(0, int(round((hi - lo) / (k_hi - k_lo))))


def _time_nc(nc, inputs_np: dict, n_iters: int = 10) -> int:
    """Min wall-clock ns over n_iters of the jitted sharded executable with
    device-resident inputs (jit'd once; donated output buffers re-placed
    untimed before each run)."""
    import time

    import jax
    from jax.experimental.shard_map import shard_map
    from jax.sharding import Mesh, PartitionSpec

    from concourse import bass2jax

    bass2jax.install_neuronx_cc_hook()

    feat_curr = np.asarray(inputs_np["feat_curr"], dtype=np.float32)
    feat_prev = np.asarray(inputs_np["feat_prev_warped"], dtype=np.float32)

    partition_name = nc.partition_id_tensor.name if nc.partition_id_tensor else None
    in_names, out_names, out_avals, zero_outs = [], [], [], []
    for alloc in nc.m.functions[0].allocations:
        if not isinstance(alloc, mybir.MemoryLocationSet):
            continue
        name = alloc.memorylocations[0].name
        if alloc.kind == "ExternalInput":
            if name != partition_name:
                in_names.append(name)
        elif alloc.kind == "ExternalOutput":
            out_names.append(name)
            shape = tuple(alloc.tensor_shape)
            dtype = mybir.dt.np(alloc.dtype)
            out_avals.append(jax.core.ShapedArray(shape, dtype))
            zero_outs.append(np.zeros(shape, dtype))
    n_params = len(in_names)
    n_outs = len(out_avals)
    in_names = in_names + out_names
    if partition_name is not None:
        in_names.append(partition_name)
    donate = tuple(range(n_params, n_params + n_outs))

    def _body(*args):
        operands = list(args)
        if partition_name is not None:
            operands.append(bass2jax.partition_id_tensor())
        outs = bass2jax._bass_exec_p.bind(
            *operands,
            out_avals=tuple(out_avals),
            in_names=tuple(in_names),
            out_names=tuple(out_names),
            lowering_input_output_aliases=(),
            sim_require_finite=True,
            sim_require_nnan=True,
            nc=nc,
        )
        return tuple(outs)

    devices = jax.devices()[:NCORES]
    mesh = Mesh(np.asarray(devices), ("core",))
    sharded = jax.jit(
        shard_map(
            _body,
            mesh=mesh,
            in_specs=(PartitionSpec("core"),) * (n_params + n_outs),
            out_specs=(PartitionSpec("core"),) * n_outs,
            check_rep=False,
        ),
        donate_argnums=donate,
        keep_unused=True,
    )
    in_map = {
        "curr": feat_curr,
        "prev": feat_prev,
        "dmask": make_dmask(),
        "ident": make_ident(),
    }
    concat_in = [
        np.concatenate(
            [
                in_map[name][c] if in_map[name].ndim == 4 else in_map[name]
                for c in range(NCORES)
            ],
            axis=0,
        )
        for name in in_names[:n_params]
    ]
    sharding = jax.sharding.NamedSharding(mesh, PartitionSpec("core"))
    dev_in = [jax.device_put(a, sharding) for a in concat_in]
    for a in dev_in:
        a.block_until_ready()

    def make_zeros():
        zs = [
            jax.device_put(
                np.zeros((NCORES * z.shape[0], *z.shape[1:]), z.dtype), sharding
            )
            for z in zero_outs
        ]
        for z in zs:
            z.block_until_ready()
        return zs

    outs = sharded(*dev_in, *make_zeros())
    for o in outs:
        o.block_until_ready()

    best = None
    for _ in range(n_iters):
        zs = make_zeros()
        t0 = time.perf_counter_ns()
        outs = sharded(*dev_in, *zs)
        for o in outs:
            o.block_until_ready()
        dt = time.perf_counter_ns() - t0
        best = dt if best is None else min(best, dt)
    return best


# revision 19
# speedup vs baseline: 1.7410x; 1.0182x over previous
"""Dilated correlation kernel for Trainium2 (8 NeuronCores, batch-parallel).

Computes, for feat_curr/feat_prev_warped [B=8, C=256, H=128, W=192] fp32:
    out[b, o, y, x] = sum_c curr_n[b,c,y,x] * prev_n[b,c,y+dy_o,x+dx_o]
over 33 (dx, dy) offsets (radius 4, dilation 2), with L2-normalized
features and zero padding outside the image.

v2 strategy (per core; batch b = core id):
  - Norms: squares (ACT Square, bf16) -> ones-matmul partition reduction
    (PE) -> sqrt (ACT, single 'sqrt' table -> no table reloads) ->
    reciprocal (DVE custom op RECIPROCAL_APPROX_FAST).
  - prev scaled by 1/||prev|| into a zero-padded bf16 plane
    [128c, ch, slot=y+4, 4+W+4]; curr kept RAW bf16 in a 16-row ring with
    16-zero gaps between 16-pixel blocks (the gaps implement half-group
    masking in the PE); curr's 1/||.|| is folded into the output stage.
  - Correlation: banded matmuls with 16-pixel half-groups. Two halves (A/B)
    of each 32-pixel PE quadrant accumulate into the SAME psum columns
    using zero-padded weights ([pix|0] / [0|pix]), so every psum partition
    sees an identically-based window and the one-hot extraction scan is
    only 16 wide. Windows: 24 cols x 5 even-dy bands, 18 cols x 2 odd-dy
    bands -> psum tile [128, 156].
  - Extraction: one-hot mask multiply (classes split DVE/Pool) + strided
    windowed add-reduce (DVE, bf16 zb).
  - Output: PE-transpose [128px, 40] -> [40, 128px]; multiply by the curr
    1/norm row (broadcast across partitions) while batching into obuf;
    contiguous [33, 1536] stores; host permutes offsets into ref order.
"""

import os
import sys

import numpy as np

_TRN_REPO = "/opt/trn_rl_repo"
if _TRN_REPO not in sys.path:
    sys.path.insert(0, _TRN_REPO)

from contextlib import ExitStack

import concourse.bacc as bacc
import concourse.bass as bass
import concourse.mybir as mybir
import concourse.tile as tile
from concourse.bass_utils import run_bass_kernel_spmd

F32 = mybir.dt.float32
BF16 = mybir.dt.bfloat16
AF = mybir.ActivationFunctionType

C, H, W = 256, 128, 192
NCORES = 8
NCHUNK = C // 128
PAD = 4
SLOT_W = W + 2 * PAD          # 200
PRING = 48                    # prev plane ring period (slots)
PECHO = 8                     # echo slots (rows with slot < PECHO double-write)
NSLOT = PRING + PECHO         # 56 physical slots
RING = 16                     # curr bf16 ring rows
G = 16                        # half-group size == one-hot scan width
EV_WIN = 24                   # even-dy band window (dy in -4,-2,0,2,4)
OD_WIN = 18                   # odd-dy band window (dy in -1,+1)
EV_COLS = 5 * EV_WIN          # 120
PS_W = EV_COLS + 2 * OD_WIN   # 156
PSTR = 1024                   # corr psum tile row pitch (two banks)
ODD_BASE = 512                # odd-band region lives in the tile's 2nd bank
NCOL = 33
OBATCH = 12
EVEN_DYS = (-4, -2, 0, 2, 4)

# flat per-partition strides
CN_PSTR = RING * NCHUNK * 12 * 32          # cn ring row stride = 768/row
PL_PSTR = NCHUNK * NSLOT * SLOT_W          # plane partition stride
PL_CH = NSLOT * SLOT_W                     # plane chunk stride

# column order produced on device (dy, dx): identical to v1
MY_OFFSETS = (
    [(dy, dx) for dy in EVEN_DYS for dx in EVEN_DYS]
    + [(dy, dx) for dy in (-1, 1) for dx in (-1, 0, 1)]
    + [(0, dx) for dx in (-1, 1)]
)


def _ref_offsets(radius=4, step=2):
    offs = []
    for dy in range(-radius, radius + 1):
        for dx in range(-radius, radius + 1):
            if abs(dx) <= 1 and abs(dy) <= 1:
                offs.append((dx, dy))
                continue
            if abs(dx) % step == 0 and abs(dy) % step == 0:
                offs.append((dx, dy))
    return offs


# perm[o_ref] = device column holding reference offset o_ref
PERM = np.array(
    [MY_OFFSETS.index((dy, dx)) for (dx, dy) in _ref_offsets()], dtype=np.int64
)


def make_dmask():
    m = np.zeros((128, G), dtype=np.float32)
    for p in range(128):
        m[p, p % G] = 1.0
    return m


def make_ident():
    return np.eye(128, dtype=np.float32)


# engine assignment knobs ('v' = DVE, 'g' = Pool/gpsimd)
# NOTE: GPSIMD (Pool) cannot read PSUM on hardware, so anything touching the
# correlation psum tile or the transpose psum tile must run on DVE (or ACT).
ENG_MULT1 = "v"
ENG_MULT23 = "v"
ENG_PREVSCALE = "g"
ENG_OBUF = "v"


def build_nc(h=H, loop_k=0):
    assert h == H
    nt = (h * W) // 128  # 192
    assert nt % OBATCH == 0
    nc = bacc.Bacc()
    curr_d = nc.declare_dram_parameter("curr", [C, h, W], F32, isOutput=False)
    prev_d = nc.declare_dram_parameter("prev", [C, h, W], F32, isOutput=False)
    mask_d = nc.declare_dram_parameter("dmask", [128, G], F32, isOutput=False)
    id_d = nc.declare_dram_parameter("ident", [128, 128], F32, isOutput=False)
    out_d = nc.declare_dram_parameter("out", [NCOL, h, W], F32, isOutput=True)

    eng = {"v": nc.vector, "g": nc.gpsimd}

    with tile.TileContext(nc) as tc, ExitStack() as ctx:
        cpool = ctx.enter_context(tc.tile_pool(name="const", bufs=1))
        ldpool = ctx.enter_context(tc.tile_pool(name="ld", bufs=2))
        sqpool = ctx.enter_context(tc.tile_pool(name="sq", bufs=2))
        snpool = ctx.enter_context(tc.tile_pool(name="sn", bufs=2))
        rnpool = ctx.enter_context(tc.tile_pool(name="rn", bufs=4))
        zbpool = ctx.enter_context(tc.tile_pool(name="zb", bufs=2))
        outpool = ctx.enter_context(tc.tile_pool(name="outp", bufs=4))
        obufpool = ctx.enter_context(tc.tile_pool(name="obuf", bufs=2))
        pscpool = ctx.enter_context(tc.tile_pool(name="psc", bufs=2, space="PSUM"))
        psnpool = ctx.enter_context(tc.tile_pool(name="psn", bufs=1, space="PSUM"))
        pstpool = ctx.enter_context(tc.tile_pool(name="pst", bufs=2, space="PSUM"))

        plane = cpool.tile([128, NCHUNK, NSLOT, SLOT_W], BF16, name="plane")
        cnr = cpool.tile([128, RING, NCHUNK, 12, 32], BF16, name="cnr")
        ones = cpool.tile([128, 128], BF16, name="ones")
        dmask = cpool.tile([128, G], F32, name="dmask")
        identf = cpool.tile([128, 128], F32, name="identf")
        identb = cpool.tile([128, 128], BF16, name="identb")
        plt = plane.tensor
        cnrt = cnr.tensor

        # zero pads of the ring plane: top pad rows -4..-1 live at slots 0..3
        # (and their echoes 48..51); left/right column pads persist forever.
        nc.gpsimd.memset(plane[:, :, 0:PAD, :], 0.0)
        nc.gpsimd.memset(plane[:, :, PRING : PRING + PAD, :], 0.0)
        nc.gpsimd.memset(plane[:, :, :, 0:PAD], 0.0)
        nc.gpsimd.memset(plane[:, :, :, SLOT_W - PAD : SLOT_W], 0.0)
        # zero gaps of the curr ring (pixel halves get overwritten; gaps persist)
        nc.gpsimd.memset(cnr[:], 0.0)
        nc.gpsimd.memset(ones[:], 1.0)
        nc.sync.dma_start(dmask[:], mask_d[:])
        nc.sync.dma_start(identf[:], id_d[:])
        nc.scalar.activation(identb[:], identf[:], AF.Copy)

        rn = {}     # (inm, strip4) -> [128, 768] f32, bcast over partitions
        state = {}

        def process_rowgroup(rg):
            ld = {}
            for ch in range(NCHUNK):
                t_ld = ldpool.tile(
                    [128, 2, 8, W], F32, name=f"ld{ch}_{rg}", tag=f"ld{ch}"
                )
                for inm, dram in ((0, curr_d), (1, prev_d)):
                    nc.sync.dma_start(
                        t_ld[:, inm], dram[ch * 128 : (ch + 1) * 128, rg : rg + 8, :]
                    )
                ld[ch] = t_ld
            # squares, both names in one op per chunk
            sq = {}
            for ch in range(NCHUNK):
                s = sqpool.tile([128, 2, 8 * W], BF16, name=f"sq{ch}_{rg}", tag=f"sq{ch}")
                nc.scalar.activation(
                    s[:],
                    ld[ch][:].rearrange("p n r w -> p n (r w)"),
                    AF.Square,
                )
                sq[ch] = s
            # norms per (name, 4-row strip): psn -> sqrt -> reciprocal
            for inm in range(2):
                for h4 in range(2):
                    s4 = rg // 4 + h4
                    # psum tile padded to a bank multiple (1024 f32 = 2 banks);
                    # matmul writes split at the 512-col bank boundary
                    psn = psnpool.tile(
                        [128, 1024], F32, name=f"psn_{inm}_{s4}", tag="psn"
                    )
                    for c0, c1 in ((0, 512), (512, 4 * W)):
                        for ch in range(NCHUNK):
                            nc.tensor.matmul(
                                psn[:, c0:c1],
                                ones[:],
                                sq[ch][:, inm, h4 * 4 * W + c0 : h4 * 4 * W + c1],
                                start=(ch == 0),
                                stop=(ch == NCHUNK - 1),
                            )
                    sn = snpool.tile([128, 4 * W], F32, name=f"sn_{inm}_{s4}", tag="sn")
                    nc.scalar.activation(sn[:], psn[:, 0 : 4 * W], AF.Sqrt)
                    rn_t = rnpool.tile(
                        [128, 4 * W], F32, name=f"rn_{inm}_{s4}", tag=f"rn{inm}"
                    )
                    nc.vector.reciprocal_approx_fast(rn_t[:], sn[:])
                    rn[(inm, s4)] = rn_t
            # prev scale into ring plane; curr raw bf16 into ring (zero-gap layout)
            for ch in range(NCHUNK):
                for h4 in range(2):
                    s4 = rg // 4 + h4
                    r0 = rg + 4 * h4
                    sP = (r0 + PAD) % PRING
                    out_ap = bass.AP(
                        plt,
                        ch * PL_CH + sP * SLOT_W + PAD,
                        [[PL_PSTR, 128], [SLOT_W, 4], [1, W]],
                    )
                    eng[ENG_PREVSCALE].tensor_mul(
                        out_ap,
                        ld[ch][:, 1, 4 * h4 : 4 * h4 + 4, :],
                        rn[(1, s4)][:].rearrange("p (r w) -> p r w", w=W),
                    )
                    if sP < PECHO:
                        # echo into slots sP+PRING so windows that wrap read
                        # a contiguous slot range
                        nc.scalar.activation(
                            bass.AP(
                                plt,
                                ch * PL_CH + (sP + PRING) * SLOT_W + PAD,
                                [[PL_PSTR, 128], [SLOT_W, 4], [1, W]],
                            ),
                            out_ap,
                            AF.Copy,
                        )
                cn_out = bass.AP(
                    cnrt,
                    (rg % RING) * NCHUNK * 384 + ch * 384 + 16,
                    [[CN_PSTR, 128], [NCHUNK * 384, 8], [32, 12], [1, 16]],
                )
                nc.scalar.activation(
                    cn_out,
                    ld[ch][:, 0].rearrange("p r (b j) -> p r b j", j=16),
                    AF.Copy,
                )

        def emit_tile(t):
            # two banks: evens accumulate in bank 0, odds in bank 1, so the
            # two psum groups can interleave (separate zero-regions)
            ps = pscpool.tile([128, 1024], F32, name=f"ps_{t}", tag="ps")
            pst = ps.tensor
            for q in range(4):
                qpix = 128 * t + 32 * q
                r, x0 = divmod(qpix, W)
                kA = x0 // 16
                sB = r % PRING
                # evens accumulate in the tile's first bank (cols 0..120),
                # odds in its second bank (cols 512..548): separate psum
                # zero-regions, so the two groups may interleave and each
                # ldweights serves both regions
                for ch in range(NCHUNK):
                    for hh in range(2):
                        lhs = bass.AP(
                            cnrt,
                            (r % RING) * NCHUNK * 384
                            + ch * 384
                            + 32 * kA
                            + 16
                            + 16 * hh,
                            [[CN_PSTR, 128], [1, 32]],
                        )
                        x0h = x0 + 16 * hh
                        first = ch == 0 and hh == 0
                        last = ch == NCHUNK - 1 and hh == 1
                        rhs_e = bass.AP(
                            plt,
                            ch * PL_CH + sB * SLOT_W + x0h,
                            [[PL_PSTR, 128], [2 * SLOT_W, 5], [1, EV_WIN]],
                        )
                        out_e = bass.AP(
                            pst, 32 * q * PSTR, [[PSTR, 32], [1, EV_COLS]]
                        )
                        nc.tensor.matmul(
                            out_e,
                            lhs,
                            rhs_e,
                            start=first,
                            stop=last,
                            tile_position=(0, 32 * q),
                        )
                        rhs_o = bass.AP(
                            plt,
                            ch * PL_CH + (sB + 3) * SLOT_W + x0h + 3,
                            [[PL_PSTR, 128], [2 * SLOT_W, 2], [1, OD_WIN]],
                        )
                        out_o = bass.AP(
                            pst,
                            32 * q * PSTR + ODD_BASE,
                            [[PSTR, 32], [1, 2 * OD_WIN]],
                        )
                        nc.tensor.matmul(
                            out_o,
                            lhs,
                            rhs_o,
                            start=first,
                            stop=last,
                            tile_position=(0, 32 * q),
                        )
            zb = zbpool.tile([128, NCOL * G], BF16, name=f"zb_{t}", tag="zb")
            zbt, dmt = zb.tensor, dmask.tensor
            SC = NCOL * G
            # class i: dy even (bands 0-4), dx even
            eng[ENG_MULT1].tensor_mul(
                bass.AP(zbt, 0, [[SC, 128], [5 * G, 5], [G, 5], [1, G]]),
                bass.AP(pst, 0, [[PSTR, 128], [EV_WIN, 5], [2, 5], [1, G]]),
                bass.AP(dmt, 0, [[G, 128], [0, 5], [0, 5], [1, G]]),
            )
            # class ii: dy in {-1,+1} (odd bands), dx in {-1,0,1}
            eng[ENG_MULT23].tensor_mul(
                bass.AP(zbt, 25 * G, [[SC, 128], [3 * G, 2], [G, 3], [1, G]]),
                bass.AP(pst, ODD_BASE, [[PSTR, 128], [OD_WIN, 2], [1, 3], [1, G]]),
                bass.AP(dmt, 0, [[G, 128], [0, 2], [0, 3], [1, G]]),
            )
            # class iii: dy=0 (band 2), dx in {-1,+1}
            eng[ENG_MULT23].tensor_mul(
                bass.AP(zbt, 31 * G, [[SC, 128], [G, 2], [1, G]]),
                bass.AP(pst, 2 * EV_WIN + 3, [[PSTR, 128], [2, 2], [1, G]]),
                bass.AP(dmt, 0, [[G, 128], [0, 2], [1, G]]),
            )
            # one-hot group reduce as a bf16 pairwise-add tree on Pool (zb is
            # SBUF so Pool may read it; at most one addend per pair is nonzero
            # so bf16 adds are exact)
            zh = zbpool.tile([128, NCOL, 8], BF16, name=f"zh_{t}", tag="zh")
            zht = zh.tensor
            outt = outpool.tile([128, 40], BF16, name=f"out_{t}", tag="outt")
            with nc.allow_low_precision("one-hot select: single-term sums"):
                nc.gpsimd.tensor_add(
                    bass.AP(zht, 0, [[NCOL * 8, 128], [8, NCOL], [1, 8]]),
                    bass.AP(zbt, 0, [[SC, 128], [G, NCOL], [1, 8]]),
                    bass.AP(zbt, 8, [[SC, 128], [G, NCOL], [1, 8]]),
                )
                nc.gpsimd.tensor_add(
                    bass.AP(zht, 4, [[NCOL * 8, 128], [8, NCOL], [1, 4]]),
                    bass.AP(zht, 0, [[NCOL * 8, 128], [8, NCOL], [1, 4]]),
                    bass.AP(zht, 4, [[NCOL * 8, 128], [8, NCOL], [1, 4]]),
                )
                nc.gpsimd.tensor_add(
                    bass.AP(zht, 6, [[NCOL * 8, 128], [8, NCOL], [1, 2]]),
                    bass.AP(zht, 4, [[NCOL * 8, 128], [8, NCOL], [1, 2]]),
                    bass.AP(zht, 6, [[NCOL * 8, 128], [8, NCOL], [1, 2]]),
                )
                nc.gpsimd.tensor_add(
                    outt[:, 0:NCOL],
                    bass.AP(zht, 6, [[NCOL * 8, 128], [8, NCOL]]),
                    bass.AP(zht, 7, [[NCOL * 8, 128], [8, NCOL]]),
                )
            k = t % OBATCH
            if k == 0:
                state["obuf"] = obufpool.tile(
                    [40, OBATCH * 128], F32, name=f"obuf_{t}", tag="obuf"
                )
            tps = pstpool.tile([40, 128], BF16, name=f"tps_{t}", tag="tps")
            nc.tensor.transpose(tps[:], outt[:], identb[:])
            # fold curr 1/norm in while batching into obuf
            s4 = t // 6
            eng[ENG_OBUF].tensor_mul(
                state["obuf"][0:NCOL, 128 * k : 128 * (k + 1)],
                tps[0:NCOL, :],
                rn[(0, s4)][0:NCOL, 128 * (t % 6) : 128 * (t % 6 + 1)],
            )
            if k == OBATCH - 1:
                t0 = t - (OBATCH - 1)
                nc.sync.dma_start(
                    bass.AP(
                        out_d,
                        128 * t0,
                        [[h * W, NCOL], [1, OBATCH * 128]],
                    ),
                    state["obuf"][0:NCOL, :],
                )

        def whole_body():
            rn.clear()
            next_t = 0
            for rg in range(0, h, 8):
                process_rowgroup(rg)
                while next_t < nt and (128 * next_t + 127) // W <= rg + 3:
                    emit_tile(next_t)
                    next_t += 1
            # bottom pad rows 128..131 at slots (128+4)%PRING..+4
            sBot = (h + PAD) % PRING
            nc.gpsimd.memset(plane[:, :, sBot : sBot + PAD, :], 0.0)
            while next_t < nt:
                emit_tile(next_t)
                next_t += 1

        if loop_k:
            with tc.For_i(0, loop_k, 1):
                whole_body()
        else:
            whole_body()

    nc.finalize()
    return nc


_NC_CACHE = {}
LAST_EXEC_NS = None


def _get_nc(h=H):
    if h not in _NC_CACHE:
        _NC_CACHE[h] = build_nc(h)
    return _NC_CACHE[h]


def kernel(feat_curr: np.ndarray, feat_prev_warped: np.ndarray) -> np.ndarray:
    global LAST_EXEC_NS
    feat_curr = np.ascontiguousarray(np.asarray(feat_curr, dtype=np.float32))
    feat_prev_warped = np.ascontiguousarray(
        np.asarray(feat_prev_warped, dtype=np.float32)
    )
    b, c, h, w = feat_curr.shape
    assert (b, c, w) == (NCORES, C, W), (b, c, w)

    nc = _get_nc(h)
    dmask = make_dmask()
    ident = make_ident()
    in_maps = [
        {
            "curr": feat_curr[i],
            "prev": feat_prev_warped[i],
            "dmask": dmask,
            "ident": ident,
        }
        for i in range(NCORES)
    ]
    res = run_bass_kernel_spmd(nc, in_maps, list(range(NCORES)))
    LAST_EXEC_NS = res.exec_time_ns
    out = np.stack([res.results[i]["out"] for i in range(NCORES)])  # [B, 33, H, W]
    out = out[:, PERM]  # reference offset order
    return np.ascontiguousarray(out)


def time_kernel(
    inputs_np: dict, n_iters: int = 10, k_lo: int = 8, k_hi: int = 136
) -> int:
    """Estimate per-iteration HW time by differencing two on-device-looped
    variants of the kernel (axon dispatch floor ~80ms makes single-shot wall
    timing useless)."""
    lo = _time_nc(build_nc(H, loop_k=k_lo), inputs_np, n_iters)
    hi = _time_nc(build_nc(H, loop_k=k_hi), inputs_np, n_iters)
    return max(0, int(round((hi - lo) / (k_hi - k_lo))))


def _time_nc(nc, inputs_np: dict, n_iters: int = 10) -> int:
    """Min wall-clock ns over n_iters of the jitted sharded executable with
    device-resident inputs (jit'd once; donated output buffers re-placed
    untimed before each run)."""
    import time

    import jax
    from jax.experimental.shard_map import shard_map
    from jax.sharding import Mesh, PartitionSpec

    from concourse import bass2jax

    bass2jax.install_neuronx_cc_hook()

    feat_curr = np.asarray(inputs_np["feat_curr"], dtype=np.float32)
    feat_prev = np.asarray(inputs_np["feat_prev_warped"], dtype=np.float32)

    partition_name = nc.partition_id_tensor.name if nc.partition_id_tensor else None
    in_names, out_names, out_avals, zero_outs = [], [], [], []
    for alloc in nc.m.functions[0].allocations:
        if not isinstance(alloc, mybir.MemoryLocationSet):
            continue
        name = alloc.memorylocations[0].name
        if alloc.kind == "ExternalInput":
            if name != partition_name:
                in_names.append(name)
        elif alloc.kind == "ExternalOutput":
            out_names.append(name)
            shape = tuple(alloc.tensor_shape)
            dtype = mybir.dt.np(alloc.dtype)
            out_avals.append(jax.core.ShapedArray(shape, dtype))
            zero_outs.append(np.zeros(shape, dtype))
    n_params = len(in_names)
    n_outs = len(out_avals)
    in_names = in_names + out_names
    if partition_name is not None:
        in_names.append(partition_name)
    donate = tuple(range(n_params, n_params + n_outs))

    def _body(*args):
        operands = list(args)
        if partition_name is not None:
            operands.append(bass2jax.partition_id_tensor())
        outs = bass2jax._bass_exec_p.bind(
            *operands,
            out_avals=tuple(out_avals),
            in_names=tuple(in_names),
            out_names=tuple(out_names),
            lowering_input_output_aliases=(),
            sim_require_finite=True,
            sim_require_nnan=True,
            nc=nc,
        )
        return tuple(outs)

    devices = jax.devices()[:NCORES]
    mesh = Mesh(np.asarray(devices), ("core",))
    sharded = jax.jit(
        shard_map(
            _body,
            mesh=mesh,
            in_specs=(PartitionSpec("core"),) * (n_params + n_outs),
            out_specs=(PartitionSpec("core"),) * n_outs,
            check_rep=False,
        ),
        donate_argnums=donate,
        keep_unused=True,
    )
    in_map = {
        "curr": feat_curr,
        "prev": feat_prev,
        "dmask": make_dmask(),
        "ident": make_ident(),
    }
    concat_in = [
        np.concatenate(
            [
                in_map[name][c] if in_map[name].ndim == 4 else in_map[name]
                for c in range(NCORES)
            ],
            axis=0,
        )
        for name in in_names[:n_params]
    ]
    sharding = jax.sharding.NamedSharding(mesh, PartitionSpec("core"))
    dev_in = [jax.device_put(a, sharding) for a in concat_in]
    for a in dev_in:
        a.block_until_ready()

    def make_zeros():
        zs = [
            jax.device_put(
                np.zeros((NCORES * z.shape[0], *z.shape[1:]), z.dtype), sharding
            )
            for z in zero_outs
        ]
        for z in zs:
            z.block_until_ready()
        return zs

    outs = sharded(*dev_in, *make_zeros())
    for o in outs:
        o.block_until_ready()

    best = None
    for _ in range(n_iters):
        zs = make_zeros()
        t0 = time.perf_counter_ns()
        outs = sharded(*dev_in, *zs)
        for o in outs:
            o.block_until_ready()
        dt = time.perf_counter_ns() - t0
        best = dt if best is None else min(best, dt)
    return best
